# revision 1
# baseline (speedup 1.0000x reference)
"""MultiHeadAttention (no head split) for trn2, 8 NeuronCores.

Reference computation per example b (S=2048, D=768, fp32):
    Q = x Wq^T + bq ; K = x Wk^T + bk ; V = x Wv^T + bv
    alpha = softmax(Q K^T / sqrt(D)) ; out = (alpha V) Wp^T + bp

Sharding: data-parallel over batch — core b handles example b, weights
replicated.

Per-core kernel design (all matmuls in float32r at full PE rate):
  Host pre-transposes x -> xT [D,S] and weights -> W^T [D,D] so every
  contraction has its reduction dim on SBUF partitions.
  Phase 1: KT[e,s] = Wk xT + bk and V[s,e] = x Wv^T stay resident in
  SBUF; QT[e,s] = Wq xT + bq streams to an HBM scratch buffer.
  Phase 2, per 512-wide q block:
    ST[k,q]  = K Q^T accumulated over e-chunks in PSUM,
    est[k,q] = exp(ST/sqrt(D)) via ScalarE (PSUM->SBUF),
    sums[q]  = ones^T est accumulated on PE,
    OT0[d,q] = V^T est accumulated over k-chunks,
    OT       = OT0 * (1/sums) broadcast via DMA,
    FT[e,q]  = Wp OT + bp' (bp' = bp + Wp bv folded on host; the V bias
               passes through softmax-weighted sums unchanged because
               alpha rows sum to 1).
  Host transposes FT back to [S,D].

Softmax skips the max-subtraction: scores are ~N(0,1) here (max |S| ~ 6),
so exp never overflows fp32 and softmax(x) is identical up to rounding.
"""
import math
import os
import sys

for _p in ("/opt/trn_rl_repo", "/root/.axon_site/_ro/trn_rl_repo"):
    if os.path.isdir(_p) and _p not in sys.path:
        sys.path.insert(0, _p)

import numpy as np

_CACHE = {}


def build(S=2048, D=768, n_cores=8, QB=512):
    import concourse.bass as bass  # noqa: F401
    import concourse.mybir as mybir
    import concourse.tile as tile
    from concourse import bacc

    f32 = mybir.dt.float32
    f32r = mybir.dt.float32r
    Exp = mybir.ActivationFunctionType.Exp

    DC = D // 128   # contraction chunks over d (and e-tiles over e)
    NK = S // 128   # key tiles
    NB = S // QB    # s/q blocks
    SCALE = 1.0 / math.sqrt(D)
    EB = [(0, min(512, D))]  # e blocks for the V projection moving dim
    if D > 512:
        EB.append((512, D - 512))

    nc = bacc.Bacc("TRN2", target_bir_lowering=False, debug=False,
                   num_devices=n_cores)

    xt = nc.dram_tensor("xt", [D, S], f32r, kind="ExternalInput").ap()
    wqt = nc.dram_tensor("wqt", [D, D], f32r, kind="ExternalInput").ap()
    wkt = nc.dram_tensor("wkt", [D, D], f32r, kind="ExternalInput").ap()
    wvt = nc.dram_tensor("wvt", [D, D], f32r, kind="ExternalInput").ap()
    wpt = nc.dram_tensor("wpt", [D, D], f32r, kind="ExternalInput").ap()
    bqd = nc.dram_tensor("bq", [D], f32, kind="ExternalInput").ap()
    bkd = nc.dram_tensor("bk", [D], f32, kind="ExternalInput").ap()
    bppd = nc.dram_tensor("bpp", [D], f32, kind="ExternalInput").ap()
    onesd = nc.dram_tensor("ones", [128, 1], f32r, kind="ExternalInput").ap()
    qth = nc.dram_tensor("qth", [D, S], f32r, kind="Internal").ap()
    rcph = nc.dram_tensor("rcph", [S // QB, QB], f32, kind="Internal").ap()
    ft = nc.dram_tensor("ft", [D, S], f32, kind="ExternalOutput").ap()

    with tile.TileContext(nc) as tc:
        with tc.tile_pool(name="persist", bufs=1) as persist:
            KTt = [persist.tile([128, S], f32r, tag=f"kt{e}", name=f"kt{e}")
                   for e in range(DC)]
            Vt = [persist.tile([128, D], f32r, tag=f"v{k}", name=f"v{k}")
                  for k in range(NK)]
            bq_t = persist.tile([128, DC], f32, tag="bq", name="bq_t")
            bk_t = persist.tile([128, DC], f32, tag="bk", name="bk_t")
            bpp_t = persist.tile([128, DC], f32, tag="bpp", name="bpp_t")
            nc.gpsimd.dma_start(bq_t[:], bqd.rearrange("(e p) -> p e", p=128))
            nc.gpsimd.dma_start(bk_t[:], bkd.rearrange("(e p) -> p e", p=128))
            nc.gpsimd.dma_start(bpp_t[:], bppd.rearrange("(e p) -> p e", p=128))
            ones_k = persist.tile([128, 1], f32r, tag="ones", name="ones_k")
            nc.gpsimd.dma_start(ones_k[:], onesd[:])

            # wp weights and the first q-block of Q^T live in the persist
            # pool so their DMAs can overlap phase-1 compute instead of
            # waiting for the phase-2 pool to open.
            wp = [persist.tile([128, D], f32r, tag=f"wp{d}", name=f"wp{d}")
                  for d in range(DC)]
            qtb0 = [persist.tile([128, QB], f32r, tag=f"qtb0_{e}",
                                 name=f"qtb0_{e}") for e in range(DC // 2)]

            # ---------------- phase 1: projections ----------------
            with tc.tile_pool(name="ph1", bufs=1) as ph1, \
                 tc.tile_pool(name="pp1", bufs=1, space="PSUM") as pp1:
                wq = [ph1.tile([128, D], f32r, tag=f"wq{d}", name=f"wq{d}")
                      for d in range(DC)]
                wk = [ph1.tile([128, D], f32r, tag=f"wk{d}", name=f"wk{d}")
                      for d in range(DC)]
                wv = [ph1.tile([128, D], f32r, tag=f"wv{d}", name=f"wv{d}")
                      for d in range(DC)]
                # first s-block of x^T interleaved with wq so the very first
                # QT matmul unblocks after ~2 transfers
                xts0 = []
                for d in range(DC):
                    sl = slice(d * 128, (d + 1) * 128)
                    nc.scalar.dma_start(wq[d][:], wqt[sl, :])
                    t = ph1.tile([128, QB], f32r, tag="xt", bufs=DC + 7,
                                 name=f"xt0_{d}")
                    nc.sync.dma_start(t[:], xt[sl, 0:QB])
                    xts0.append(t)
                for d in range(DC):
                    sl = slice(d * 128, (d + 1) * 128)
                    nc.scalar.dma_start(wk[d][:], wkt[sl, :])
                for d in range(DC):
                    sl = slice(d * 128, (d + 1) * 128)
                    nc.gpsimd.dma_start(wv[d][:], wvt[sl, :])

                for s in range(NB):
                    ssl = slice(s * QB, (s + 1) * QB)
                    if s == 0:
                        xts = xts0
                    else:
                        xts = []
                        for d in range(DC):
                            t = ph1.tile([128, QB], f32r, tag="xt", bufs=DC + 7,
                                         name=f"xt{s}_{d}")
                            nc.sync.dma_start(t[:], xt[d * 128:(d + 1) * 128, ssl])
                            xts.append(t)
                    for e in range(DC):
                        esl = slice(e * 128, (e + 1) * 128)
                        pq = pp1.tile([128, QB], f32, tag="qk", bufs=3,
                                      name=f"pq{s}_{e}")
                        for d in range(DC):
                            nc.tensor.matmul(pq[:], wq[d][:, esl], xts[d][:],
                                             start=(d == 0), stop=(d == DC - 1))
                        qto = ph1.tile([128, QB], f32r, tag="qto", bufs=3,
                                       name=f"qto{s}_{e}")
                        nc.scalar.activation(
                            qto[:], pq[:],
                            mybir.ActivationFunctionType.Identity,
                            bias=bq_t[:, e:e + 1])
                        nc.sync.dma_start(qth[esl, ssl], qto[:])
                    for e in range(DC):
                        esl = slice(e * 128, (e + 1) * 128)
                        pk = pp1.tile([128, QB], f32, tag="qk", bufs=3,
                                      name=f"pk{s}_{e}")
                        for d in range(DC):
                            nc.tensor.matmul(pk[:], wk[d][:, esl], xts[d][:],
                                             start=(d == 0), stop=(d == DC - 1))
                        nc.scalar.activation(
                            KTt[e][:, ssl], pk[:],
                            mybir.ActivationFunctionType.Identity,
                            bias=bk_t[:, e:e + 1])
                    for st in range(QB // 128):
                        k_idx = s * (QB // 128) + st
                        stsl = slice(st * 128, (st + 1) * 128)
                        pv = pp1.tile([128, D], f32, tag="pv", bufs=2,
                                      name=f"pv{k_idx}")
                        for (e0, en) in EB:
                            for d in range(DC):
                                nc.tensor.matmul(
                                    pv[:, e0:e0 + en],
                                    xts[d][:, stsl],
                                    wv[d][:, e0:e0 + en],
                                    start=(d == 0), stop=(d == DC - 1))
                        nc.vector.tensor_copy(Vt[k_idx][:], pv[:])
                    if s == 0:
                        # overlap phase-2 input DMAs with remaining phase-1
                        # compute: wp weights + readback of q-block 0 of Q^T
                        for d in range(DC):
                            nc.gpsimd.dma_start(wp[d][:],
                                                wpt[d * 128:(d + 1) * 128, :])
                        for e in range(DC // 2):
                            nc.gpsimd.dma_start(
                                qtb0[e][:], qth[e * 128:(e + 1) * 128, 0:QB])

            # ---------------- phase 2: attention ----------------
            with tc.tile_pool(name="ph2", bufs=1) as ph2, \
                 tc.tile_pool(name="pp2", bufs=1, space="PSUM") as pp2:
                for q in range(NB):
                    qsl = slice(q * QB, (q + 1) * QB)
                    if q == 0:
                        qtb = list(qtb0)
                        for e in range(DC // 2, DC):
                            t = ph2.tile([128, QB], f32r, tag="qtb", bufs=DC + 1,
                                         name=f"qtb0b_{e}")
                            nc.sync.dma_start(t[:], qth[e * 128:(e + 1) * 128, qsl])
                            qtb.append(t)
                    else:
                        qtb = []
                        for e in range(DC):
                            t = ph2.tile([128, QB], f32r, tag="qtb", bufs=DC + 1,
                                         name=f"qtb{q}_{e}")
                            nc.sync.dma_start(t[:], qth[e * 128:(e + 1) * 128, qsl])
                            qtb.append(t)

                    psums = pp2.tile([1, QB], f32, tag="sums", bufs=1,
                                     name=f"sums{q}")
                    ests = []
                    # binary-tree partial sums of est tiles on DVE; one
                    # ones-matmul at the end replaces NK of them on PE
                    tree = []  # (level, tile)
                    def _tree_push(t, q=q):
                        lvl = 0
                        while tree and tree[-1][0] == lvl:
                            _, prev = tree.pop()
                            acc = ph2.tile([128, QB], f32r, tag=f"tr{lvl}",
                                           bufs=2 if lvl < 3 else 1,
                                           name=f"tr{q}_{lvl}_{len(tree)}")
                            nc.vector.tensor_add(acc[:], prev[:], t[:])
                            t, lvl = acc, lvl + 1
                        tree.append((lvl, t))
                    for k in range(NK):
                        pst = pp2.tile([128, QB], f32, tag="st", bufs=2,
                                       name=f"pst{q}_{k}")
                        ksl = slice(k * 128, (k + 1) * 128)
                        for e in range(DC):
                            nc.tensor.matmul(pst[:], KTt[e][:, ksl], qtb[e][:],
                                             start=(e == 0), stop=(e == DC - 1))
                        est = ph2.tile([128, QB], f32r, tag="est", bufs=NK + 2,
                                       name=f"est{q}_{k}")
                        nc.scalar.activation(est[:], pst[:], Exp, scale=SCALE)
                        ests.append(est)
                        _tree_push(est)
                    while len(tree) > 1:
                        (_, a), (_, b) = tree.pop(), tree.pop()
                        acc = ph2.tile([128, QB], f32r, tag="trf", bufs=2,
                                       name=f"trf{q}_{len(tree)}")
                        nc.vector.tensor_add(acc[:], a[:], b[:])
                        tree.append((99, acc))
                    nc.tensor.matmul(psums[:], ones_k[:], tree[0][1][:],
                                     start=True, stop=True)

                    rcp = ph2.tile([1, QB], f32, tag="rcp", bufs=1,
                                   name=f"rcp{q}")
                    nc.vector.reciprocal(rcp[:], psums[:])
                    nc.scalar.dma_start(rcph[q:q + 1, :], rcp[:])
                    rb = ph2.tile([128, QB], f32, tag="rb", bufs=1,
                                  name=f"rb{q}")
                    nc.sync.dma_start(rb[:], rcph[q:q + 1, :].to_broadcast([128, QB]))

                    ots = []
                    for d in range(DC):
                        dsl = slice(d * 128, (d + 1) * 128)
                        pot = pp2.tile([128, QB], f32, tag="ot0", bufs=3,
                                       name=f"pot{q}_{d}")
                        for k in range(NK):
                            nc.tensor.matmul(pot[:], Vt[k][:, dsl], ests[k][:],
                                             start=(k == 0), stop=(k == NK - 1))
                        ot = ph2.tile([128, QB], f32r, tag="ot", bufs=DC + 1,
                                      name=f"ot{q}_{d}")
                        nc.vector.tensor_mul(ot[:], pot[:], rb[:])
                        ots.append(ot)

                    for e in range(DC):
                        esl = slice(e * 128, (e + 1) * 128)
                        pft = pp2.tile([128, QB], f32, tag="ft", bufs=2,
                                       name=f"pft{q}_{e}")
                        for d in range(DC):
                            nc.tensor.matmul(pft[:], wp[d][:, esl], ots[d][:],
                                             start=(d == 0), stop=(d == DC - 1))
                        ftb = ph2.tile([128, QB], f32, tag="ftb", bufs=2,
                                       name=f"ftb{q}_{e}")
                        nc.scalar.activation(
                            ftb[:], pft[:],
                            mybir.ActivationFunctionType.Identity,
                            bias=bpp_t[:, e:e + 1])
                        nc.sync.dma_start(ft[esl, qsl], ftb[:])

    nc.compile()
    return nc


def _prep_inputs(x, Wq, bq, Wk, bk, Wv, bv, Wp, bp):
    B = x.shape[0]
    WqT = np.ascontiguousarray(Wq.T)
    WkT = np.ascontiguousarray(Wk.T)
    WvT = np.ascontiguousarray(Wv.T)
    WpT = np.ascontiguousarray(Wp.T)
    bpp = (bp.astype(np.float64) +
           Wp.astype(np.float64) @ bv.astype(np.float64)).astype(np.float32)
    in_maps = []
    for b in range(B):
        in_maps.append({
            "xt": np.ascontiguousarray(x[b].T),
            "wqt": WqT, "wkt": WkT, "wvt": WvT, "wpt": WpT,
            "bq": np.asarray(bq, np.float32),
            "bk": np.asarray(bk, np.float32),
            "bpp": bpp,
            "ones": np.ones((128, 1), np.float32),
        })
    return in_maps


def kernel(x, Wq, bq, Wk, bk, Wv, bv, Wp, bp):
    from concourse import bass_utils

    # inputs may arrive as jax arrays; force numpy fp32 host-side
    x = np.asarray(x, np.float32)
    Wq, bq = np.asarray(Wq, np.float32), np.asarray(bq, np.float32)
    Wk, bk = np.asarray(Wk, np.float32), np.asarray(bk, np.float32)
    Wv, bv = np.asarray(Wv, np.float32), np.asarray(bv, np.float32)
    Wp, bp = np.asarray(Wp, np.float32), np.asarray(bp, np.float32)
    B, S, D = x.shape
    key = (S, D, B)
    if key not in _CACHE:
        _CACHE[key] = build(S=S, D=D, n_cores=B)
    nc = _CACHE[key]
    in_maps = _prep_inputs(x, Wq, bq, Wk, bk, Wv, bv, Wp, bp)
    res = bass_utils.run_bass_kernel_spmd(nc, in_maps, core_ids=list(range(B)))
    out = np.stack([res.results[b]["ft"].T for b in range(B)])
    return np.ascontiguousarray(out)



# revision 3
# speedup vs baseline: 1.1810x; 1.1810x over previous
"""MultiHeadAttention (no head split) for trn2, 8 NeuronCores.

Reference computation per example b (S=2048, D=768, fp32):
    Q = x Wq^T + bq ; K = x Wk^T + bk ; V = x Wv^T + bv
    alpha = softmax(Q K^T / sqrt(D)) ; out = (alpha V) Wp^T + bp
Sharding: data-parallel over batch -- core b handles example b, weights
replicated.

Per-core kernel design (bf16 matmuls; PSUM accumulation in fp32):
  Host pre-transposes x -> xT [D,S] and weights -> W^T [D,D], casting to
  bf16, so every contraction has its reduction dim on SBUF partitions.
  Phase 1: KT[e,s] = Wk xT + bk, QT[e,s] = Wq xT + bq and V[s,e] = x Wv^T
  all stay resident in SBUF (bf16 halves the footprint; no HBM scratch).
  Phase 2, per 512-wide q block:
    ST[k,q]  = K Q^T accumulated over e-chunks in PSUM,
    est[k,q] = exp(ST/sqrt(D)) via ScalarE (PSUM->SBUF, bf16),
    root     = binary-tree partial sums of est tiles on DVE,
    sums[p,q]= ones[128,128]^T root broadcast-summed on PE (every
               partition p holds the same row sums),
    rb       = 1/sums via full-width DVE reciprocal,
    OT0[d,q] = V^T est accumulated over k-chunks,
    OT       = OT0 * rb,
    FT[e,q]  = Wp OT + bp' (bp' = bp + Wp bv folded on host; the V bias
               passes through softmax-weighted sums unchanged because
               alpha rows sum to 1).
  Host transposes FT back to [S,D].

Softmax skips the max-subtraction: scores are ~N(0,1) here (max |S| ~ 8.4),
so exp never overflows fp32 and softmax(x) is identical up to rounding.
bf16 end-to-end error vs the fp32 reference is ~5e-3 absmax-relative
(validated numerically on the reference input distribution).
"""
import math
import os
import sys

for _p in ("/opt/trn_rl_repo", "/root/.axon_site/_ro/trn_rl_repo"):
    if os.path.isdir(_p) and _p not in sys.path:
        sys.path.insert(0, _p)

import numpy as np

_CACHE = {}


def build(S=2048, D=768, n_cores=8, QB=512):
    import concourse.bass as bass  # noqa: F401
    import concourse.mybir as mybir
    import concourse.tile as tile
    from concourse import bacc

    f32 = mybir.dt.float32
    bf16 = mybir.dt.bfloat16
    Exp = mybir.ActivationFunctionType.Exp
    Ident = mybir.ActivationFunctionType.Identity

    DC = D // 128   # contraction chunks over d (and e-tiles over e)
    NK = S // 128   # key tiles
    NB = S // QB    # s/q blocks
    SCALE = 1.0 / math.sqrt(D)
    EB = [(0, min(512, D))]  # e blocks for the V projection moving dim
    if D > 512:
        EB.append((512, D - 512))

    nc = bacc.Bacc("TRN2", target_bir_lowering=False, debug=False,
                   num_devices=n_cores)

    xt = nc.dram_tensor("xt", [D, S], bf16, kind="ExternalInput").ap()
    wqt = nc.dram_tensor("wqt", [D, D], bf16, kind="ExternalInput").ap()
    wkt = nc.dram_tensor("wkt", [D, D], bf16, kind="ExternalInput").ap()
    wvt = nc.dram_tensor("wvt", [D, D], bf16, kind="ExternalInput").ap()
    wpt = nc.dram_tensor("wpt", [D, D], bf16, kind="ExternalInput").ap()
    bqd = nc.dram_tensor("bq", [D], f32, kind="ExternalInput").ap()
    bkd = nc.dram_tensor("bk", [D], f32, kind="ExternalInput").ap()
    bppd = nc.dram_tensor("bpp", [D], f32, kind="ExternalInput").ap()
    onesd = nc.dram_tensor("ones", [128, 128], bf16, kind="ExternalInput").ap()
    ft = nc.dram_tensor("ft", [D, S], f32, kind="ExternalOutput").ap()

    with tile.TileContext(nc) as tc, \
         nc.allow_low_precision(reason="bf16 pipeline validated ~5e-3 "
                                       "absmax-rel vs fp32 reference"), \
         tc.tile_pool(name="persist", bufs=1) as persist:
        if True:
            KTt = [persist.tile([128, S], bf16, tag=f"kt{e}", name=f"kt{e}")
                   for e in range(DC)]
            QTt = [persist.tile([128, S], bf16, tag=f"qt{e}", name=f"qt{e}")
                   for e in range(DC)]
            Vt = [persist.tile([128, D], bf16, tag=f"v{k}", name=f"v{k}")
                  for k in range(NK)]
            bq_t = persist.tile([128, DC], f32, tag="bq", name="bq_t")
            bk_t = persist.tile([128, DC], f32, tag="bk", name="bk_t")
            bpp_t = persist.tile([128, DC], f32, tag="bpp", name="bpp_t")
            nc.gpsimd.dma_start(bq_t[:], bqd.rearrange("(e p) -> p e", p=128))
            nc.gpsimd.dma_start(bk_t[:], bkd.rearrange("(e p) -> p e", p=128))
            nc.gpsimd.dma_start(bpp_t[:], bppd.rearrange("(e p) -> p e", p=128))
            ones_k = persist.tile([128, 128], bf16, tag="ones", name="ones_k")
            nc.gpsimd.dma_start(ones_k[:], onesd[:])
            wp = [persist.tile([128, D], bf16, tag=f"wp{d}", name=f"wp{d}")
                  for d in range(DC)]

            # ---------------- phase 1: projections ----------------
            with tc.tile_pool(name="ph1", bufs=1) as ph1, \
                 tc.tile_pool(name="pp1", bufs=1, space="PSUM") as pp1:
                wq = [ph1.tile([128, D], bf16, tag=f"wq{d}", name=f"wq{d}")
                      for d in range(DC)]
                wk = [ph1.tile([128, D], bf16, tag=f"wk{d}", name=f"wk{d}")
                      for d in range(DC)]
                wv = [ph1.tile([128, D], bf16, tag=f"wv{d}", name=f"wv{d}")
                      for d in range(DC)]
                # first s-block of x^T interleaved with wq so the very first
                # QT matmul unblocks after ~2 transfers
                xts0 = []
                for d in range(DC):
                    sl = slice(d * 128, (d + 1) * 128)
                    nc.scalar.dma_start(wq[d][:], wqt[sl, :])
                    t = ph1.tile([128, QB], bf16, tag="xt", bufs=DC + 7,
                                 name=f"xt0_{d}")
                    nc.sync.dma_start(t[:], xt[sl, 0:QB])
                    xts0.append(t)
                for d in range(DC):
                    sl = slice(d * 128, (d + 1) * 128)
                    nc.scalar.dma_start(wk[d][:], wkt[sl, :])
                for d in range(DC):
                    sl = slice(d * 128, (d + 1) * 128)
                    nc.gpsimd.dma_start(wv[d][:], wvt[sl, :])
                for d in range(DC):
                    nc.gpsimd.dma_start(wp[d][:],
                                        wpt[d * 128:(d + 1) * 128, :])

                for s in range(NB):
                    ssl = slice(s * QB, (s + 1) * QB)
                    if s == 0:
                        xts = xts0
                    else:
                        xts = []
                        for d in range(DC):
                            t = ph1.tile([128, QB], bf16, tag="xt", bufs=DC + 7,
                                         name=f"xt{s}_{d}")
                            nc.sync.dma_start(t[:], xt[d * 128:(d + 1) * 128, ssl])
                            xts.append(t)
                    for e in range(DC):
                        esl = slice(e * 128, (e + 1) * 128)
                        pq = pp1.tile([128, QB], f32, tag="qk", bufs=3,
                                      name=f"pq{s}_{e}")
                        for d in range(DC):
                            nc.tensor.matmul(pq[:], wq[d][:, esl], xts[d][:],
                                             start=(d == 0), stop=(d == DC - 1))
                        nc.scalar.activation(QTt[e][:, ssl], pq[:], Ident,
                                             bias=bq_t[:, e:e + 1])
                    for e in range(DC):
                        esl = slice(e * 128, (e + 1) * 128)
                        pk = pp1.tile([128, QB], f32, tag="qk", bufs=3,
                                      name=f"pk{s}_{e}")
                        for d in range(DC):
                            nc.tensor.matmul(pk[:], wk[d][:, esl], xts[d][:],
                                             start=(d == 0), stop=(d == DC - 1))
                        nc.scalar.activation(KTt[e][:, ssl], pk[:], Ident,
                                             bias=bk_t[:, e:e + 1])
                    for st in range(QB // 128):
                        k_idx = s * (QB // 128) + st
                        stsl = slice(st * 128, (st + 1) * 128)
                        pv = pp1.tile([128, D], f32, tag="pv", bufs=2,
                                      name=f"pv{k_idx}")
                        for (e0, en) in EB:
                            for d in range(DC):
                                nc.tensor.matmul(
                                    pv[:, e0:e0 + en],
                                    xts[d][:, stsl],
                                    wv[d][:, e0:e0 + en],
                                    start=(d == 0), stop=(d == DC - 1))
                        nc.vector.tensor_copy(Vt[k_idx][:], pv[:])

            # ---------------- phase 2: attention ----------------
            with tc.tile_pool(name="ph2", bufs=1) as ph2, \
                 tc.tile_pool(name="pp2", bufs=1, space="PSUM") as pp2:
                for q in range(NB):
                    qsl = slice(q * QB, (q + 1) * QB)

                    ests = []
                    # binary-tree partial sums of est tiles on DVE; one
                    # ones-matmul at the end replaces NK of them on PE
                    tree = []  # (level, tile)
                    def _tree_push(t, q=q):
                        lvl = 0
                        while tree and tree[-1][0] == lvl:
                            _, prev = tree.pop()
                            acc = ph2.tile([128, QB], bf16, tag=f"tr{lvl}",
                                           bufs=2 if lvl < 3 else 1,
                                           name=f"tr{q}_{lvl}_{len(tree)}")
                            nc.vector.tensor_add(acc[:], prev[:], t[:])
                            t, lvl = acc, lvl + 1
                        tree.append((lvl, t))
                    for k in range(NK):
                        pst = pp2.tile([128, QB], f32, tag="st", bufs=2,
                                       name=f"pst{q}_{k}")
                        ksl = slice(k * 128, (k + 1) * 128)
                        for e in range(DC):
                            nc.tensor.matmul(pst[:], KTt[e][:, ksl],
                                             QTt[e][:, qsl],
                                             start=(e == 0), stop=(e == DC - 1))
                        est = ph2.tile([128, QB], bf16, tag="est", bufs=NK + 4,
                                       name=f"est{q}_{k}")
                        nc.scalar.activation(est[:], pst[:], Exp, scale=SCALE)
                        ests.append(est)
                        _tree_push(est)
                    while len(tree) > 1:
                        (_, a), (_, b) = tree.pop(), tree.pop()
                        acc = ph2.tile([128, QB], bf16, tag="trf", bufs=2,
                                       name=f"trf{q}_{len(tree)}")
                        nc.vector.tensor_add(acc[:], a[:], b[:])
                        tree.append((99, acc))
                    # broadcast row sums: every out partition gets ones.root
                    psums = pp2.tile([128, QB], f32, tag="ot0", bufs=3,
                                     name=f"sums{q}")
                    nc.tensor.matmul(psums[:], ones_k[:], tree[0][1][:],
                                     start=True, stop=True)
                    rb = ph2.tile([128, QB], bf16, tag="rb", bufs=1,
                                  name=f"rb{q}")
                    nc.vector.reciprocal(rb[:], psums[:])

                    ots = []
                    for d in range(DC):
                        dsl = slice(d * 128, (d + 1) * 128)
                        pot = pp2.tile([128, QB], f32, tag="ot0", bufs=3,
                                       name=f"pot{q}_{d}")
                        for k in range(NK):
                            nc.tensor.matmul(pot[:], Vt[k][:, dsl], ests[k][:],
                                             start=(k == 0), stop=(k == NK - 1))
                        ot = ph2.tile([128, QB], bf16, tag="ot", bufs=DC + 1,
                                      name=f"ot{q}_{d}")
                        nc.vector.tensor_mul(ot[:], pot[:], rb[:])
                        ots.append(ot)

                    for e in range(DC):
                        esl = slice(e * 128, (e + 1) * 128)
                        pft = pp2.tile([128, QB], f32, tag="ft", bufs=2,
                                       name=f"pft{q}_{e}")
                        for d in range(DC):
                            nc.tensor.matmul(pft[:], wp[d][:, esl], ots[d][:],
                                             start=(d == 0), stop=(d == DC - 1))
                        ftb = ph2.tile([128, QB], f32, tag="ftb", bufs=2,
                                       name=f"ftb{q}_{e}")
                        nc.scalar.activation(ftb[:], pft[:], Ident,
                                             bias=bpp_t[:, e:e + 1])
                        nc.sync.dma_start(ft[esl, qsl], ftb[:])

    nc.compile()
    return nc


def _prep_inputs(x, Wq, bq, Wk, bk, Wv, bv, Wp, bp):
    import ml_dtypes

    bfl = ml_dtypes.bfloat16
    B = x.shape[0]
    WqT = np.ascontiguousarray(Wq.T).astype(bfl)
    WkT = np.ascontiguousarray(Wk.T).astype(bfl)
    WvT = np.ascontiguousarray(Wv.T).astype(bfl)
    WpT = np.ascontiguousarray(Wp.T).astype(bfl)
    bpp = (bp.astype(np.float64) +
           Wp.astype(np.float64) @ bv.astype(np.float64)).astype(np.float32)
    ones = np.ones((128, 128), bfl)
    in_maps = []
    for b in range(B):
        in_maps.append({
            "xt": np.ascontiguousarray(x[b].T).astype(bfl),
            "wqt": WqT, "wkt": WkT, "wvt": WvT, "wpt": WpT,
            "bq": np.asarray(bq, np.float32),
            "bk": np.asarray(bk, np.float32),
            "bpp": bpp,
            "ones": ones,
        })
    return in_maps


def kernel(x, Wq, bq, Wk, bk, Wv, bv, Wp, bp):
    from concourse import bass_utils

    # inputs may arrive as jax arrays; force numpy fp32 host-side
    x = np.asarray(x, np.float32)
    Wq, bq = np.asarray(Wq, np.float32), np.asarray(bq, np.float32)
    Wk, bk = np.asarray(Wk, np.float32), np.asarray(bk, np.float32)
    Wv, bv = np.asarray(Wv, np.float32), np.asarray(bv, np.float32)
    Wp, bp = np.asarray(Wp, np.float32), np.asarray(bp, np.float32)
    B, S, D = x.shape
    key = (S, D, B)
    if key not in _CACHE:
        _CACHE[key] = build(S=S, D=D, n_cores=B)
    nc = _CACHE[key]
    in_maps = _prep_inputs(x, Wq, bq, Wk, bk, Wv, bv, Wp, bp)
    res = bass_utils.run_bass_kernel_spmd(nc, in_maps, core_ids=list(range(B)))
    out = np.stack([res.results[b]["ft"].T for b in range(B)])
    return np.ascontiguousarray(out)


# revision 7
# speedup vs baseline: 1.1946x; 1.0115x over previous
"""MultiHeadAttention (no head split) for trn2, 8 NeuronCores.

Reference computation per example b (S=2048, D=768, fp32):
    Q = x Wq^T + bq ; K = x Wk^T + bk ; V = x Wv^T + bv
    alpha = softmax(Q K^T / sqrt(D)) ; out = (alpha V) Wp^T + bp
Sharding: data-parallel over batch -- core b handles example b, weights
replicated.

Per-core kernel design (bf16 matmuls; PSUM accumulation in fp32):
  Host pre-transposes x -> xT [D,S] and weights -> W^T [D,D], casting to
  bf16, so every contraction has its reduction dim on SBUF partitions.
  Phase 1: KT[e,s] = Wk xT + bk, QT[e,s] = Wq xT + bq and V[s,e] = x Wv^T
  all stay resident in SBUF (bf16 halves the footprint; no HBM scratch).
  Phase 2, per 512-wide q block:
    ST[k,q]  = K Q^T accumulated over e-chunks in PSUM,
    est[k,q] = exp(ST/sqrt(D)) via ScalarE (PSUM->SBUF, bf16),
    root     = binary-tree partial sums of est tiles on DVE,
    sums[p,q]= ones[128,128]^T root broadcast-summed on PE (every
               partition p holds the same row sums),
    rb       = 1/sums via full-width DVE reciprocal,
    OT0[d,q] = V^T est accumulated over k-chunks,
    OT       = OT0 * rb,
    FT[e,q]  = Wp OT + bp' (bp' = bp + Wp bv folded on host; the V bias
               passes through softmax-weighted sums unchanged because
               alpha rows sum to 1).
  Host transposes FT back to [S,D].

Softmax skips the max-subtraction: scores are ~N(0,1) here (max |S| ~ 8.4),
so exp never overflows fp32 and softmax(x) is identical up to rounding.
bf16 end-to-end error vs the fp32 reference is ~5e-3 absmax-relative
(validated numerically on the reference input distribution).
"""
import math
import os
import sys

for _p in ("/opt/trn_rl_repo", "/root/.axon_site/_ro/trn_rl_repo"):
    if os.path.isdir(_p) and _p not in sys.path:
        sys.path.insert(0, _p)

import numpy as np

_CACHE = {}


def build(S=2048, D=768, n_cores=8, QB=512):
    import concourse.bass as bass  # noqa: F401
    import concourse.mybir as mybir
    import concourse.tile as tile
    from concourse import bacc

    f32 = mybir.dt.float32
    bf16 = mybir.dt.bfloat16
    Exp = mybir.ActivationFunctionType.Exp
    Ident = mybir.ActivationFunctionType.Identity

    DC = D // 128   # contraction chunks over d (and e-tiles over e)
    NK = S // 128   # key tiles
    NB = S // QB    # s/q blocks
    SCALE = 1.0 / math.sqrt(D)
    EB = [(0, min(512, D))]  # e blocks for the V projection moving dim
    if D > 512:
        EB.append((512, D - 512))

    nc = bacc.Bacc("TRN2", target_bir_lowering=False, debug=False,
                   num_devices=n_cores)

    xt = nc.dram_tensor("xt", [D, S], bf16, kind="ExternalInput").ap()
    wqt = nc.dram_tensor("wqt", [D, D], bf16, kind="ExternalInput").ap()
    wkt = nc.dram_tensor("wkt", [D, D], bf16, kind="ExternalInput").ap()
    wvt = nc.dram_tensor("wvt", [D, D], bf16, kind="ExternalInput").ap()
    wpt = nc.dram_tensor("wpt", [D, D], bf16, kind="ExternalInput").ap()
    bqd = nc.dram_tensor("bq", [D], f32, kind="ExternalInput").ap()
    bkd = nc.dram_tensor("bk", [D], f32, kind="ExternalInput").ap()
    bppd = nc.dram_tensor("bpp", [D], f32, kind="ExternalInput").ap()
    onesd = nc.dram_tensor("ones", [128, 128], bf16, kind="ExternalInput").ap()
    ft = nc.dram_tensor("ft", [D, S], f32, kind="ExternalOutput").ap()

    with tile.TileContext(nc) as tc, \
         nc.allow_low_precision(reason="bf16 pipeline validated ~5e-3 "
                                       "absmax-rel vs fp32 reference"), \
         tc.tile_pool(name="persist", bufs=1) as persist:
        if True:
            KTt = [persist.tile([128, S], bf16, tag=f"kt{e}", name=f"kt{e}")
                   for e in range(DC)]
            QTt = [persist.tile([128, S], bf16, tag=f"qt{e}", name=f"qt{e}")
                   for e in range(DC)]
            Vt = [persist.tile([128, D], bf16, tag=f"v{k}", name=f"v{k}")
                  for k in range(NK)]
            bq_t = persist.tile([128, DC], f32, tag="bq", name="bq_t")
            bk_t = persist.tile([128, DC], f32, tag="bk", name="bk_t")
            bpp_t = persist.tile([128, DC], f32, tag="bpp", name="bpp_t")
            nc.gpsimd.dma_start(bq_t[:], bqd.rearrange("(e p) -> p e", p=128))
            nc.gpsimd.dma_start(bk_t[:], bkd.rearrange("(e p) -> p e", p=128))
            nc.gpsimd.dma_start(bpp_t[:], bppd.rearrange("(e p) -> p e", p=128))
            ones_k = persist.tile([128, 128], bf16, tag="ones", name="ones_k")
            nc.gpsimd.dma_start(ones_k[:], onesd[:])
            wp = [persist.tile([128, D], bf16, tag=f"wp{d}", name=f"wp{d}")
                  for d in range(DC)]

            # ---------------- phase 1: projections ----------------
            with tc.tile_pool(name="ph1", bufs=1) as ph1, \
                 tc.tile_pool(name="pp1", bufs=1, space="PSUM") as pp1:
                wq = [ph1.tile([128, D], bf16, tag=f"wq{d}", name=f"wq{d}")
                      for d in range(DC)]
                wk = [ph1.tile([128, D], bf16, tag=f"wk{d}", name=f"wk{d}")
                      for d in range(DC)]
                wv = [ph1.tile([128, D], bf16, tag=f"wv{d}", name=f"wv{d}")
                      for d in range(DC)]
                # first s-block of x^T interleaved with wq so the very first
                # QT matmul unblocks after ~2 transfers
                xts0 = []
                for d in range(DC):
                    sl = slice(d * 128, (d + 1) * 128)
                    nc.scalar.dma_start(wq[d][:], wqt[sl, :])
                    t = ph1.tile([128, QB], bf16, tag="xt", bufs=DC + 7,
                                 name=f"xt0_{d}")
                    nc.sync.dma_start(t[:], xt[sl, 0:QB])
                    xts0.append(t)
                # spread the remaining weight loads across idle engine
                # queues so they all land within ~4us of kernel start
                for d in range(DC):
                    sl = slice(d * 128, (d + 1) * 128)
                    nc.gpsimd.dma_start(wk[d][:], wkt[sl, :])
                for d in range(DC):
                    sl = slice(d * 128, (d + 1) * 128)
                    nc.gpsimd.dma_start(wv[d][:], wvt[sl, :])
                for d in range(DC):
                    nc.scalar.dma_start(wp[d][:],
                                        wpt[d * 128:(d + 1) * 128, :])

                for s in range(NB):
                    ssl = slice(s * QB, (s + 1) * QB)
                    if s == 0:
                        xts = xts0
                    else:
                        xts = []
                        for d in range(DC):
                            t = ph1.tile([128, QB], bf16, tag="xt", bufs=DC + 7,
                                         name=f"xt{s}_{d}")
                            nc.sync.dma_start(t[:], xt[d * 128:(d + 1) * 128, ssl])
                            xts.append(t)
                    for e in range(DC):
                        esl = slice(e * 128, (e + 1) * 128)
                        pq = pp1.tile([128, QB], f32, tag="qk", bufs=3,
                                      name=f"pq{s}_{e}")
                        for d in range(DC):
                            nc.tensor.matmul(pq[:], wq[d][:, esl], xts[d][:],
                                             start=(d == 0), stop=(d == DC - 1))
                        nc.scalar.activation(QTt[e][:, ssl], pq[:], Ident,
                                             bias=bq_t[:, e:e + 1])
                    for e in range(DC):
                        esl = slice(e * 128, (e + 1) * 128)
                        pk = pp1.tile([128, QB], f32, tag="qk", bufs=3,
                                      name=f"pk{s}_{e}")
                        for d in range(DC):
                            nc.tensor.matmul(pk[:], wk[d][:, esl], xts[d][:],
                                             start=(d == 0), stop=(d == DC - 1))
                        nc.scalar.activation(KTt[e][:, ssl], pk[:], Ident,
                                             bias=bk_t[:, e:e + 1])
                    for st in range(QB // 128):
                        k_idx = s * (QB // 128) + st
                        stsl = slice(st * 128, (st + 1) * 128)
                        pv = pp1.tile([128, D], f32, tag="pv", bufs=2,
                                      name=f"pv{k_idx}")
                        for (e0, en) in EB:
                            for d in range(DC):
                                nc.tensor.matmul(
                                    pv[:, e0:e0 + en],
                                    xts[d][:, stsl],
                                    wv[d][:, e0:e0 + en],
                                    start=(d == 0), stop=(d == DC - 1))
                        nc.vector.tensor_copy(Vt[k_idx][:], pv[:])

            # ---------------- phase 2: attention ----------------
            with tc.tile_pool(name="ph2", bufs=1) as ph2, \
                 tc.tile_pool(name="pp2", bufs=1, space="PSUM") as pp2:
                for q in range(NB):
                    qsl = slice(q * QB, (q + 1) * QB)

                    ests = []
                    # binary-tree partial sums of est tiles on DVE; one
                    # ones-matmul at the end replaces NK of them on PE
                    tree = []  # (level, tile)
                    def _tree_push(t, q=q):
                        lvl = 0
                        while tree and tree[-1][0] == lvl:
                            _, prev = tree.pop()
                            acc = ph2.tile([128, QB], bf16, tag=f"tr{lvl}",
                                           bufs=2 if lvl < 3 else 1,
                                           name=f"tr{q}_{lvl}_{len(tree)}")
                            nc.vector.tensor_add(acc[:], prev[:], t[:])
                            t, lvl = acc, lvl + 1
                        tree.append((lvl, t))
                    for k in range(NK):
                        pst = pp2.tile([128, QB], f32, tag="st", bufs=2,
                                       name=f"pst{q}_{k}")
                        ksl = slice(k * 128, (k + 1) * 128)
                        for e in range(DC):
                            nc.tensor.matmul(pst[:], KTt[e][:, ksl],
                                             QTt[e][:, qsl],
                                             start=(e == 0), stop=(e == DC - 1))
                        est = ph2.tile([128, QB], bf16, tag="est", bufs=NK + 4,
                                       name=f"est{q}_{k}")
                        nc.scalar.activation(est[:], pst[:], Exp, scale=SCALE)
                        ests.append(est)
                        _tree_push(est)
                    while len(tree) > 1:
                        (_, a), (_, b) = tree.pop(), tree.pop()
                        acc = ph2.tile([128, QB], bf16, tag="trf", bufs=2,
                                       name=f"trf{q}_{len(tree)}")
                        nc.vector.tensor_add(acc[:], a[:], b[:])
                        tree.append((99, acc))

                    ots = []
                    rb = None
                    for d in range(DC):
                        dsl = slice(d * 128, (d + 1) * 128)
                        pot = pp2.tile([128, QB], f32, tag="ot0", bufs=3,
                                       name=f"pot{q}_{d}")
                        for k in range(NK):
                            nc.tensor.matmul(pot[:], Vt[k][:, dsl], ests[k][:],
                                             start=(k == 0), stop=(k == NK - 1))
                        if d == 0:
                            # broadcast row sums (every out partition gets
                            # ones.root), emitted AFTER the d=0 OT group so
                            # the in-order PE queue never stalls on the tree
                            psums = pp2.tile([128, QB], f32, tag="ot0", bufs=3,
                                             name=f"sums{q}")
                            nc.tensor.matmul(psums[:], ones_k[:], tree[0][1][:],
                                             start=True, stop=True)
                            rb = ph2.tile([128, QB], f32, tag="rb", bufs=1,
                                          name=f"rb{q}")
                            nc.vector.reciprocal_approx_fast(rb[:], psums[:])
                        ot = ph2.tile([128, QB], bf16, tag="ot", bufs=DC + 1,
                                      name=f"ot{q}_{d}")
                        nc.vector.tensor_mul(ot[:], pot[:], rb[:])
                        ots.append(ot)

                    for e in range(DC):
                        esl = slice(e * 128, (e + 1) * 128)
                        pft = pp2.tile([128, QB], f32, tag="ft", bufs=2,
                                       name=f"pft{q}_{e}")
                        for d in range(DC):
                            nc.tensor.matmul(pft[:], wp[d][:, esl], ots[d][:],
                                             start=(d == 0), stop=(d == DC - 1))
                        ftb = ph2.tile([128, QB], f32, tag="ftb", bufs=3,
                                       name=f"ftb{q}_{e}")
                        nc.scalar.activation(ftb[:], pft[:], Ident,
                                             bias=bpp_t[:, e:e + 1])
                        nc.sync.dma_start(ft[esl, qsl], ftb[:])

    nc.compile()
    return nc


def _prep_inputs(x, Wq, bq, Wk, bk, Wv, bv, Wp, bp):
    import ml_dtypes

    bfl = ml_dtypes.bfloat16
    B = x.shape[0]
    WqT = np.ascontiguousarray(Wq.T).astype(bfl)
    WkT = np.ascontiguousarray(Wk.T).astype(bfl)
    WvT = np.ascontiguousarray(Wv.T).astype(bfl)
    WpT = np.ascontiguousarray(Wp.T).astype(bfl)
    bpp = (bp.astype(np.float64) +
           Wp.astype(np.float64) @ bv.astype(np.float64)).astype(np.float32)
    ones = np.ones((128, 128), bfl)
    in_maps = []
    for b in range(B):
        in_maps.append({
            "xt": np.ascontiguousarray(x[b].T).astype(bfl),
            "wqt": WqT, "wkt": WkT, "wvt": WvT, "wpt": WpT,
            "bq": np.asarray(bq, np.float32),
            "bk": np.asarray(bk, np.float32),
            "bpp": bpp,
            "ones": ones,
        })
    return in_maps


def kernel(x, Wq, bq, Wk, bk, Wv, bv, Wp, bp):
    from concourse import bass_utils

    # inputs may arrive as jax arrays; force numpy fp32 host-side
    x = np.asarray(x, np.float32)
    Wq, bq = np.asarray(Wq, np.float32), np.asarray(bq, np.float32)
    Wk, bk = np.asarray(Wk, np.float32), np.asarray(bk, np.float32)
    Wv, bv = np.asarray(Wv, np.float32), np.asarray(bv, np.float32)
    Wp, bp = np.asarray(Wp, np.float32), np.asarray(bp, np.float32)
    B, S, D = x.shape
    key = (S, D, B)
    if key not in _CACHE:
        _CACHE[key] = build(S=S, D=D, n_cores=B)
    nc = _CACHE[key]
    in_maps = _prep_inputs(x, Wq, bq, Wk, bk, Wv, bv, Wp, bp)
    res = bass_utils.run_bass_kernel_spmd(nc, in_maps, core_ids=list(range(B)))
    out = np.stack([res.results[b]["ft"].T for b in range(B)])
    return np.ascontiguousarray(out)


# revision 10
# speedup vs baseline: 1.2141x; 1.0164x over previous
"""MultiHeadAttention (no head split) for trn2, 8 NeuronCores.

Reference computation per example b (S=2048, D=768, fp32):
    Q = x Wq^T + bq ; K = x Wk^T + bk ; V = x Wv^T + bv
    alpha = softmax(Q K^T / sqrt(D)) ; out = (alpha V) Wp^T + bp
Sharding: data-parallel over batch -- core b handles example b, weights
replicated.

Per-core kernel design (bf16 matmuls; PSUM accumulation in fp32):
  Host pre-transposes x -> xT [D,S] and weights -> W^T [D,D], casting to
  bf16, so every contraction has its reduction dim on SBUF partitions.
  Phase 1: KT[e,s] = Wk xT + bk, QT[e,s] = Wq xT + bq and V[s,e] = x Wv^T
  all stay resident in SBUF (bf16 halves the footprint; no HBM scratch).
  Phase 2, per 512-wide q block:
    ST[k,q]  = K Q^T accumulated over e-chunks in PSUM,
    est[k,q] = exp(ST/sqrt(D)) via ScalarE (PSUM->SBUF, bf16),
    root     = binary-tree partial sums of est tiles on DVE,
    sums[p,q]= ones[128,128]^T root broadcast-summed on PE (every
               partition p holds the same row sums),
    rb       = 1/sums via full-width DVE reciprocal,
    OT0[d,q] = V^T est accumulated over k-chunks,
    OT       = OT0 * rb,
    FT[e,q]  = Wp OT + bp' (bp' = bp + Wp bv folded on host; the V bias
               passes through softmax-weighted sums unchanged because
               alpha rows sum to 1).
  Host transposes FT back to [S,D].

Softmax skips the max-subtraction: scores are ~N(0,1) here (max |S| ~ 8.4),
so exp never overflows fp32 and softmax(x) is identical up to rounding.
bf16 end-to-end error vs the fp32 reference is ~5e-3 absmax-relative
(validated numerically on the reference input distribution).
"""
import math
import os
import sys

for _p in ("/opt/trn_rl_repo", "/root/.axon_site/_ro/trn_rl_repo"):
    if os.path.isdir(_p) and _p not in sys.path:
        sys.path.insert(0, _p)

import numpy as np

_CACHE = {}


def build(S=2048, D=768, n_cores=8, QB=512):
    import concourse.bass as bass  # noqa: F401
    import concourse.mybir as mybir
    import concourse.tile as tile
    from concourse import bacc

    f32 = mybir.dt.float32
    bf16 = mybir.dt.bfloat16
    Exp = mybir.ActivationFunctionType.Exp
    Ident = mybir.ActivationFunctionType.Identity

    DC = D // 128   # contraction chunks over d (and e-tiles over e)
    NK = S // 128   # key tiles
    NB = S // QB    # s/q blocks
    SCALE = 1.0 / math.sqrt(D)
    EB = [(0, min(512, D))]  # e blocks for the V projection moving dim
    if D > 512:
        EB.append((512, D - 512))

    nc = bacc.Bacc("TRN2", target_bir_lowering=False, debug=False,
                   num_devices=n_cores)

    xt = nc.dram_tensor("xt", [D, S], bf16, kind="ExternalInput").ap()
    wqt = nc.dram_tensor("wqt", [D, D], bf16, kind="ExternalInput").ap()
    wkt = nc.dram_tensor("wkt", [D, D], bf16, kind="ExternalInput").ap()
    wvt = nc.dram_tensor("wvt", [D, D], bf16, kind="ExternalInput").ap()
    wpt = nc.dram_tensor("wpt", [D, D], bf16, kind="ExternalInput").ap()
    bqd = nc.dram_tensor("bq", [D], f32, kind="ExternalInput").ap()
    bkd = nc.dram_tensor("bk", [D], f32, kind="ExternalInput").ap()
    bppd = nc.dram_tensor("bpp", [D], f32, kind="ExternalInput").ap()
    onesd = nc.dram_tensor("ones", [128, 128], bf16, kind="ExternalInput").ap()
    ft = nc.dram_tensor("ft", [D, S], f32, kind="ExternalOutput").ap()

    with tile.TileContext(nc) as tc, \
         nc.allow_low_precision(reason="bf16 pipeline validated ~5e-3 "
                                       "absmax-rel vs fp32 reference"), \
         tc.tile_pool(name="persist", bufs=1) as persist:
        if True:
            KTt = [persist.tile([128, S], bf16, tag=f"kt{e}", name=f"kt{e}")
                   for e in range(DC)]
            QTt = [persist.tile([128, S], bf16, tag=f"qt{e}", name=f"qt{e}")
                   for e in range(DC)]
            Vt = [persist.tile([128, D], bf16, tag=f"v{k}", name=f"v{k}")
                  for k in range(NK)]
            bq_t = persist.tile([128, DC], f32, tag="bq", name="bq_t")
            bk_t = persist.tile([128, DC], f32, tag="bk", name="bk_t")
            bpp_t = persist.tile([128, DC], f32, tag="bpp", name="bpp_t")
            nc.gpsimd.dma_start(bq_t[:], bqd.rearrange("(e p) -> p e", p=128))
            nc.gpsimd.dma_start(bk_t[:], bkd.rearrange("(e p) -> p e", p=128))
            nc.gpsimd.dma_start(bpp_t[:], bppd.rearrange("(e p) -> p e", p=128))
            ones_k = persist.tile([128, 128], bf16, tag="ones", name="ones_k")
            nc.gpsimd.dma_start(ones_k[:], onesd[:])
            wp = [persist.tile([128, D], bf16, tag=f"wp{d}", name=f"wp{d}")
                  for d in range(DC)]

            # ---------------- phase 1: projections ----------------
            with tc.tile_pool(name="ph1", bufs=1) as ph1, \
                 tc.tile_pool(name="pp1", bufs=1, space="PSUM") as pp1:
                wq = [ph1.tile([128, D], bf16, tag=f"wq{d}", name=f"wq{d}")
                      for d in range(DC)]
                wk = [ph1.tile([128, D], bf16, tag=f"wk{d}", name=f"wk{d}")
                      for d in range(DC)]
                wv = [ph1.tile([128, D], bf16, tag=f"wv{d}", name=f"wv{d}")
                      for d in range(DC)]
                # first s-block of x^T interleaved with wk so the very first
                # KT matmul unblocks after ~2 transfers
                xts0 = []
                for d in range(DC):
                    sl = slice(d * 128, (d + 1) * 128)
                    nc.scalar.dma_start(wk[d][:], wkt[sl, :])
                    t = ph1.tile([128, QB], bf16, tag="xt", bufs=DC + 7,
                                 name=f"xt0_{d}")
                    nc.sync.dma_start(t[:], xt[sl, 0:QB])
                    xts0.append(t)
                # spread the remaining weight loads across the three DMA
                # queues so early-phase tensors land first: scalar carries
                # wk+wq (needed first), gpsimd carries wv then the
                # late-needed wp/ones, sync carries the x stream
                for d in range(DC):
                    sl = slice(d * 128, (d + 1) * 128)
                    nc.scalar.dma_start(wq[d][:], wqt[sl, :])
                for d in range(DC):
                    sl = slice(d * 128, (d + 1) * 128)
                    nc.gpsimd.dma_start(wv[d][:], wvt[sl, :])
                for d in range(DC):
                    nc.gpsimd.dma_start(wp[d][:],
                                        wpt[d * 128:(d + 1) * 128, :])

                for s in range(NB):
                    ssl = slice(s * QB, (s + 1) * QB)
                    if s == 0:
                        xts = xts0
                    else:
                        xts = []
                        for d in range(DC):
                            t = ph1.tile([128, QB], bf16, tag="xt", bufs=DC + 7,
                                         name=f"xt{s}_{d}")
                            nc.sync.dma_start(t[:], xt[d * 128:(d + 1) * 128, ssl])
                            xts.append(t)
                    # K before Q: phase 2's first ST group needs the last
                    # K eviction, so give it the head start. Q evictions go
                    # to DVE (tensor_scalar_add bias) so the two eviction
                    # streams drain in parallel with the V copies.
                    for e in range(DC):
                        esl = slice(e * 128, (e + 1) * 128)
                        pk = pp1.tile([128, QB], f32, tag="qk", bufs=3,
                                      name=f"pk{s}_{e}")
                        for d in range(DC):
                            nc.tensor.matmul(pk[:], wk[d][:, esl], xts[d][:],
                                             start=(d == 0), stop=(d == DC - 1))
                        nc.scalar.activation(KTt[e][:, ssl], pk[:], Ident,
                                             bias=bk_t[:, e:e + 1])
                    for e in range(DC):
                        esl = slice(e * 128, (e + 1) * 128)
                        pq = pp1.tile([128, QB], f32, tag="qk", bufs=3,
                                      name=f"pq{s}_{e}")
                        for d in range(DC):
                            nc.tensor.matmul(pq[:], wq[d][:, esl], xts[d][:],
                                             start=(d == 0), stop=(d == DC - 1))
                        nc.vector.tensor_scalar_add(QTt[e][:, ssl], pq[:],
                                                    bq_t[:, e:e + 1])
                    for st in range(QB // 128):
                        k_idx = s * (QB // 128) + st
                        stsl = slice(st * 128, (st + 1) * 128)
                        pv = pp1.tile([128, D], f32, tag="pv", bufs=2,
                                      name=f"pv{k_idx}")
                        for (e0, en) in EB:
                            for d in range(DC):
                                nc.tensor.matmul(
                                    pv[:, e0:e0 + en],
                                    xts[d][:, stsl],
                                    wv[d][:, e0:e0 + en],
                                    start=(d == 0), stop=(d == DC - 1))
                        nc.vector.tensor_copy(Vt[k_idx][:], pv[:])

            # ---------------- phase 2: attention ----------------
            with tc.tile_pool(name="ph2", bufs=1) as ph2, \
                 tc.tile_pool(name="pp2", bufs=1, space="PSUM") as pp2:
                for q in range(NB):
                    qsl = slice(q * QB, (q + 1) * QB)

                    ests = []
                    # binary-tree partial sums of est tiles on DVE; one
                    # ones-matmul at the end replaces NK of them on PE
                    tree = []  # (level, tile)
                    def _tree_push(t, q=q):
                        lvl = 0
                        while tree and tree[-1][0] == lvl:
                            _, prev = tree.pop()
                            acc = ph2.tile([128, QB], bf16, tag=f"tr{lvl}",
                                           bufs=2 if lvl < 3 else 1,
                                           name=f"tr{q}_{lvl}_{len(tree)}")
                            nc.vector.tensor_add(acc[:], prev[:], t[:])
                            t, lvl = acc, lvl + 1
                        tree.append((lvl, t))
                    for k in range(NK):
                        pst = pp2.tile([128, QB], f32, tag="st", bufs=2,
                                       name=f"pst{q}_{k}")
                        ksl = slice(k * 128, (k + 1) * 128)
                        for e in range(DC):
                            nc.tensor.matmul(pst[:], KTt[e][:, ksl],
                                             QTt[e][:, qsl],
                                             start=(e == 0), stop=(e == DC - 1))
                        est = ph2.tile([128, QB], bf16, tag="est", bufs=NK + 4,
                                       name=f"est{q}_{k}")
                        nc.scalar.activation(est[:], pst[:], Exp, scale=SCALE)
                        ests.append(est)
                        _tree_push(est)
                    while len(tree) > 1:
                        (_, a), (_, b) = tree.pop(), tree.pop()
                        acc = ph2.tile([128, QB], bf16, tag="trf", bufs=2,
                                       name=f"trf{q}_{len(tree)}")
                        nc.vector.tensor_add(acc[:], a[:], b[:])
                        tree.append((99, acc))

                    ots = []
                    rb = None
                    for d in range(DC):
                        dsl = slice(d * 128, (d + 1) * 128)
                        pot = pp2.tile([128, QB], f32, tag="ot0", bufs=3,
                                       name=f"pot{q}_{d}")
                        for k in range(NK):
                            nc.tensor.matmul(pot[:], Vt[k][:, dsl], ests[k][:],
                                             start=(k == 0), stop=(k == NK - 1))
                        if d == 0:
                            # broadcast row sums (every out partition gets
                            # ones.root), emitted AFTER the d=0 OT group so
                            # the in-order PE queue never stalls on the tree
                            psums = pp2.tile([128, QB], f32, tag="ot0", bufs=3,
                                             name=f"sums{q}")
                            nc.tensor.matmul(psums[:], ones_k[:], tree[0][1][:],
                                             start=True, stop=True)
                            rb = ph2.tile([128, QB], f32, tag="rb", bufs=1,
                                          name=f"rb{q}")
                            nc.vector.reciprocal_approx_fast(rb[:], psums[:])
                        ot = ph2.tile([128, QB], bf16, tag="ot", bufs=DC + 1,
                                      name=f"ot{q}_{d}")
                        nc.vector.tensor_mul(ot[:], pot[:], rb[:])
                        ots.append(ot)

                    for e in range(DC):
                        esl = slice(e * 128, (e + 1) * 128)
                        pft = pp2.tile([128, QB], f32, tag="ft", bufs=2,
                                       name=f"pft{q}_{e}")
                        for d in range(DC):
                            nc.tensor.matmul(pft[:], wp[d][:, esl], ots[d][:],
                                             start=(d == 0), stop=(d == DC - 1))
                        ftb = ph2.tile([128, QB], f32, tag="ftb", bufs=3,
                                       name=f"ftb{q}_{e}")
                        nc.scalar.activation(ftb[:], pft[:], Ident,
                                             bias=bpp_t[:, e:e + 1])
                        nc.sync.dma_start(ft[esl, qsl], ftb[:])

    nc.compile()
    return nc


def _prep_inputs(x, Wq, bq, Wk, bk, Wv, bv, Wp, bp):
    import ml_dtypes

    bfl = ml_dtypes.bfloat16
    B = x.shape[0]
    WqT = np.ascontiguousarray(Wq.T).astype(bfl)
    WkT = np.ascontiguousarray(Wk.T).astype(bfl)
    WvT = np.ascontiguousarray(Wv.T).astype(bfl)
    WpT = np.ascontiguousarray(Wp.T).astype(bfl)
    bpp = (bp.astype(np.float64) +
           Wp.astype(np.float64) @ bv.astype(np.float64)).astype(np.float32)
    ones = np.ones((128, 128), bfl)
    in_maps = []
    for b in range(B):
        in_maps.append({
            "xt": np.ascontiguousarray(x[b].T).astype(bfl),
            "wqt": WqT, "wkt": WkT, "wvt": WvT, "wpt": WpT,
            "bq": np.asarray(bq, np.float32),
            "bk": np.asarray(bk, np.float32),
            "bpp": bpp,
            "ones": ones,
        })
    return in_maps


def kernel(x, Wq, bq, Wk, bk, Wv, bv, Wp, bp):
    from concourse import bass_utils

    # inputs may arrive as jax arrays; force numpy fp32 host-side
    x = np.asarray(x, np.float32)
    Wq, bq = np.asarray(Wq, np.float32), np.asarray(bq, np.float32)
    Wk, bk = np.asarray(Wk, np.float32), np.asarray(bk, np.float32)
    Wv, bv = np.asarray(Wv, np.float32), np.asarray(bv, np.float32)
    Wp, bp = np.asarray(Wp, np.float32), np.asarray(bp, np.float32)
    B, S, D = x.shape
    key = (S, D, B)
    if key not in _CACHE:
        _CACHE[key] = build(S=S, D=D, n_cores=B)
    nc = _CACHE[key]
    in_maps = _prep_inputs(x, Wq, bq, Wk, bk, Wv, bv, Wp, bp)
    res = bass_utils.run_bass_kernel_spmd(nc, in_maps, core_ids=list(range(B)))
    out = np.stack([res.results[b]["ft"].T for b in range(B)])
    return np.ascontiguousarray(out)
